# revision 22
# baseline (speedup 1.0000x reference)
"""Multi-head cross-attention kernel for 8 Trainium2 NeuronCores.

Problem (nn_Attention): B=2, F=T=2048, H=1024, N=16 heads, D=64.
    q = query @ wq;  k = source @ wk;  v = source @ wv     ([B,L,N,D])
    logits = (q * D^-0.5) . k  (+ bias);  w = softmax(logits, T)
    out = (w . v) @ wo                                      ([B,F,H])

Sharding: 8 cores = 2 (batch) x 4 (head groups of 4 heads). Each core
computes its batch's partial output over its 4 heads; the host sums the
4 per-group partials per batch (output projection is linear in heads).

v2 design notes (vs the fp32r baseline):
  - All matmul operands are bf16 (cost model: 1.0 cycles/output-row for
    any N, vs fp32r which needs N>=256).  Accuracy measured ~0.5% absmax
    end-to-end, well under the 2e-2 gate.
  - Logits matmuls run with K=64 directly (cost is independent of K), so
    no zero-padded wq and Q projection is pair-packed: 32768 cycles
    instead of 65536.
  - PV runs in [f-partition, (head,d)-free] orientation: per (head,
    t-tile, f-tile) the matmul is lhsT=pt-slice [128t x 128f], rhs =
    v|1 [128t x 65] -> ctx[128f, 65] in PSUM.  66.5K cycles vs 131K for
    the [d, f] orientation (output rows are full 128).
  - ctx is normalized (per-partition scalar = 1/den from the ones
    column), transposed back to [(h2,d), f] via PE transposes, and the
    output projection accumulates ctxT^T @ wo pairs into [128f, 512h]
    PSUM tiles DMA'd straight to DRAM.
  - exp runs on ScalarE from PSUM [128, 2, 512] tiles (~1.07us each,
    128 per core).  To keep ScalarE busy from ~8us on, logits+exp for
    the NEXT f-block run ahead (pt tiles retained in a 34-deep SBUF
    ring) while PV accumulation stays f-block-major (PSUM ctx capacity
    allows only one f-block's accumulators).

bias is all-zero for this problem (spec fill=zeros); a nonzero bias falls
back to a numpy reference implementation for correctness.
"""

import numpy as np

B, F, T, H, NH, D = 2, 2048, 2048, 1024, 16, 64
NCORES = 8
GROUPS = 4           # head groups (one per core within a batch)
HPG = NH // GROUPS   # 4 heads per core
PAIRS = HPG // 2     # head pairs per core
P = 128

_CACHE = {}


def _build_nc(F_=F, T_=T, H_=H, loop=1, sim_trace=False, skip_compile=False,
              debug=False):
    """Build the per-core Bass program. All 8 cores run this same program
    on different input data. loop>1 repeats the whole body inside the NEFF
    (benchmarking aid: isolates HW time from dispatch overhead)."""
    import concourse.bass as bass  # noqa: F401  (registers engine types)
    import concourse.mybir as mybir
    from concourse import bacc
    from concourse.tile import TileContext

    f32 = mybir.dt.float32
    bf16 = mybir.dt.bfloat16

    nc = bacc.Bacc("TRN2", target_bir_lowering=False, debug=False,
                   num_devices=NCORES)

    qT_d = nc.dram_tensor("qT", [H_, F_], bf16, kind="ExternalInput")
    sT_d = nc.dram_tensor("sT", [H_, T_], bf16, kind="ExternalInput")
    wq_d = nc.dram_tensor("wq", [H_, HPG * D], bf16, kind="ExternalInput")
    wk_d = nc.dram_tensor("wk", [H_, HPG * D], bf16, kind="ExternalInput")
    wv_d = nc.dram_tensor("wv", [H_, HPG * D], bf16, kind="ExternalInput")
    wo_d = nc.dram_tensor("wo", [P, PAIRS, H_], bf16, kind="ExternalInput")
    id_d = nc.dram_tensor("ident", [P, P], bf16, kind="ExternalInput")
    out_d = nc.dram_tensor("out", [F_, H_], bf16, kind="ExternalOutput")

    env = dict(H_=H_, F_=F_, T_=T_, qT_d=qT_d, sT_d=sT_d, wq_d=wq_d,
               wk_d=wk_d, wv_d=wv_d, wo_d=wo_d, id_d=id_d, out_d=out_d,
               debug=debug)
    if debug:
        bf16 = mybir.dt.bfloat16
        env["dbg_qblk"] = nc.dram_tensor("dbg_qblk", [P, PAIRS, F_], bf16,
                                         kind="ExternalOutput")
        env["dbg_kTp"] = nc.dram_tensor("dbg_kTp", [P, PAIRS, T_], bf16,
                                        kind="ExternalOutput")
        env["dbg_vplus"] = nc.dram_tensor("dbg_vplus",
                                          [P, T_ // P, HPG, D + 1], bf16,
                                          kind="ExternalOutput")
        env["dbg_ctxT"] = nc.dram_tensor("dbg_ctxT", [P, PAIRS, F_], bf16,
                                         kind="ExternalOutput")
        env["dbg_ctxsb"] = nc.dram_tensor("dbg_ctxsb",
                                          [F_ // 512, P, 4, HPG, D], bf16,
                                          kind="ExternalOutput")

    with TileContext(nc, trace_sim=sim_trace) as tc:
        with (
            tc.tile_pool(name="weights", bufs=1) as wpool,
            tc.tile_pool(name="persist", bufs=1) as perspool,
            tc.tile_pool(name="stream", bufs=2) as streampool,
            tc.tile_pool(name="ptlive", bufs=4) as ptlive,
            tc.tile_pool(name="ptring", bufs=34) as ptring,
            tc.tile_pool(name="nrm", bufs=2) as nrmpool,
            tc.tile_pool(name="osb", bufs=3) as osbpool,
            tc.tile_pool(name="small", bufs=2) as smallpool,
            tc.tile_pool(name="ps_s", bufs=2, space="PSUM") as ps_s,
            tc.tile_pool(name="ps_ctx", bufs=1, space="PSUM") as ps_ctx,
            tc.tile_pool(name="ps_tail", bufs=1, space="PSUM") as ps_tail,
        ):
            env.update(wpool=wpool, perspool=perspool, streampool=streampool,
                       ptlive=ptlive, ptring=ptring, nrmpool=nrmpool,
                       osbpool=osbpool, smallpool=smallpool, ps_s=ps_s,
                       ps_ctx=ps_ctx, ps_tail=ps_tail)
            import contextlib
            loop_ctx = tc.For_i(0, loop, 1) if loop > 1 else contextlib.nullcontext()
            with loop_ctx:
                _emit_body(nc, tc, env)

    if not skip_compile:
        nc.compile()
    return nc


def _emit_body(nc, tc, env):
    import concourse.mybir as mybir
    f32 = mybir.dt.float32
    bf16 = mybir.dt.bfloat16
    AF = mybir.ActivationFunctionType
    (H_, F_, T_) = (env[k] for k in ("H_", "F_", "T_"))
    (qT_d, sT_d, wq_d, wk_d, wv_d, wo_d, id_d, out_d) = (env[k] for k in
        ("qT_d", "sT_d", "wq_d", "wk_d", "wv_d", "wo_d", "id_d", "out_d"))
    (wpool, perspool, streampool, ptlive, ptring, nrmpool, osbpool,
     smallpool, ps_s, ps_ctx, ps_tail) = (env[k] for k in
        ("wpool", "perspool", "streampool", "ptlive", "ptring", "nrmpool",
         "osbpool", "smallpool", "ps_s", "ps_ctx", "ps_tail"))

    HT = H_ // P          # contraction tiles for projections (8)
    FB = F_ // 512        # f-blocks (4)
    TB = T_ // 512        # source chunks (4)
    TT = T_ // P          # t-tiles (16)

    qT_v = qT_d[:].rearrange("(o p) f -> p o f", p=P)   # [128, HT, F]
    sT_v = sT_d[:].rearrange("(o p) f -> p o f", p=P)
    wq_v = wq_d[:].rearrange("(o p) c -> p o c", p=P)   # [128, HT, 256]
    wk_v = wk_d[:].rearrange("(o p) c -> p o c", p=P)
    wv_v = wv_d[:].rearrange("(o p) c -> p o c", p=P)

    # ---- persistent SBUF tensors ----
    wq_sb = wpool.tile([P, HT, HPG * D], bf16)   # cols = (pair, h2, d)
    wk_sb = wpool.tile([P, HT, HPG * D], bf16)
    wv_sb = wpool.tile([P, HT, HPG * D], bf16)   # cols = (head, d)
    wo_sb = wpool.tile([P, PAIRS, H_], bf16)     # rows = (h2, d)
    id_sb = wpool.tile([P, P], bf16)

    qblk = perspool.tile([P, PAIRS, F_], bf16)   # pair-packed q^T
    kTp = perspool.tile([P, PAIRS, T_], bf16)    # pair-packed k^T
    vplus = perspool.tile([P, TT, HPG, D + 1], bf16)  # [t%128, tt, h, d|1]
    ctxT = perspool.tile([P, PAIRS, F_], bf16)   # normalized ctx^T

    nc.vector.memset(vplus[:, :, :, D:D + 1], 1.0)

    nc.sync.dma_start(wq_sb[:], wq_v)

    def proj_chunk(src_v, w_sb, dst, c):
        """Project one 512-col chunk of qT/sT through wq/wk (pair-packed
        M=128 output tiles) into dst[:, :, c*512:(c+1)*512]."""
        chunk = streampool.tile([P, HT, 512], bf16, tag="chunk", name="chunk")
        nc.sync.dma_start(chunk[:], src_v[:, :, c * 512:(c + 1) * 512])
        ps = ps_s.tile([P, 2, 512], f32, tag="s", name="ps_proj")
        for pair in range(PAIRS):
            for ht in range(HT):
                nc.tensor.matmul(
                    ps[:, pair, :],
                    w_sb[:, ht, pair * P:(pair + 1) * P],
                    chunk[:, ht, :],
                    start=(ht == 0), stop=(ht == HT - 1),
                )
        nc.vector.tensor_copy(dst[:, :, c * 512:(c + 1) * 512], ps[:])
        return chunk

    def vproj_chunk(schunk, tb):
        """sourceT chunk -> v tiles [t, (h,d)] written into vplus."""
        ps = ps_s.tile([P, 2, 512], f32, tag="s", name="ps_v")
        for tc4 in range(4):
            for ht in range(HT):
                nc.tensor.matmul(
                    ps[:, tc4 // 2, (tc4 % 2) * 256:(tc4 % 2 + 1) * 256],
                    schunk[:, ht, tc4 * P:(tc4 + 1) * P],
                    wv_sb[:, ht, :],
                    start=(ht == 0), stop=(ht == HT - 1),
                )
        nc.vector.tensor_copy(
            vplus[:, tb * 4:(tb + 1) * 4, :, 0:D],
            ps[:].rearrange("p a (b h d) -> p (a b) h d", b=2, h=HPG),
        )

    def unit_exp(fb, tp, head, pool):
        """Logits (2 t-tiles) + exp for one (f-block, t-pair, head).
        Returns the pt tile holding exp(S^T) [128t, 2, 512f]."""
        pair, h2 = divmod(head, 2)
        lo = 64 * h2
        s = ps_s.tile([P, 2, 512], f32, tag="s", name="s")
        for i in range(2):
            tt = 2 * tp + i
            nc.tensor.matmul(
                s[:, i, :],
                kTp[lo:lo + 64, pair, tt * P:(tt + 1) * P],
                qblk[lo:lo + 64, pair, fb * 512:(fb + 1) * 512],
                start=True, stop=True,
            )
        pt = pool.tile([P, 2, 512], bf16, tag="pt", name="pt")
        nc.scalar.activation(pt[:], s[:], AF.Exp)
        return pt

    def unit_pv(ctx_ps, pt, tp, head):
        """Accumulate pt into this f-block's ctx PSUM for one unit.

        start is never set: 16 accumulation groups (ft x head) share the
        ctx PSUM banks, and a start_tensor_calc marks its whole 2KB
        ZERO_REGION pending-zero, wiping other groups' partials.  The
        tile is DVE-memset to zero once per f-block instead."""
        for i in range(2):
            tt = 2 * tp + i
            for ft in range(4):
                nc.tensor.matmul(
                    ctx_ps[:, ft, head, :],
                    pt[:, i, ft * P:(ft + 1) * P],
                    vplus[:, tt, head, :],
                    start=False, stop=(tt == TT - 1),
                    skip_group_check=True,
                )

    def tail_ops(fb, ctx_ps):
        """Normalize + transpose + output projection for a finished
        f-block. DVE normalize is emitted inline; returns a list of
        closures (PE transposes / oproj / copies) to interleave into the
        next phase so single-buffer PSUM reuse stalls overlap real work."""
        recip = smallpool.tile([P, 4, HPG, 1], f32, tag="recip", name="recip")
        nc.vector.reciprocal(recip[:], ctx_ps[:, :, :, D:D + 1])
        ctx_sb = nrmpool.tile([P, 4, HPG, D], bf16, tag="nrm", name="ctx_sb")
        for ft in range(4):
            for h in range(HPG):
                nc.vector.tensor_scalar_mul(
                    ctx_sb[:, ft, h, :],
                    ctx_ps[:, ft, h, 0:D],
                    recip[:, ft, h, :],
                )
        if env.get("debug"):
            nc.sync.dma_start(env["dbg_ctxsb"][fb], ctx_sb[:])
        ops = []

        def transp(ft):
            tl = ps_tail.tile([P, 512], f32, tag="tail", name="tp_ps")
            tlb = tl.bitcast(bf16).rearrange("p (a b) -> p a b", a=2)
            for h in range(HPG):
                pr, h2 = divmod(h, 2)
                nc.tensor.transpose(
                    tlb[64 * h2:64 * (h2 + 1), pr, 0:P],
                    ctx_sb[:, ft, h, :],
                    id_sb[:],
                    tile_position=(0, 64 * h2),
                )
            nc.vector.tensor_copy(
                ctxT[:, :, (fb * 4 + ft) * P:(fb * 4 + ft + 1) * P],
                tlb[:, :, 0:P],
            )

        def oproj(ft, hb):
            o = ps_tail.tile([P, 512], f32, tag="tail", name="oproj")
            for pr in range(PAIRS):
                nc.tensor.matmul(
                    o[:],
                    ctxT[:, pr, (fb * 4 + ft) * P:(fb * 4 + ft + 1) * P],
                    wo_sb[:, pr, hb * 512:(hb + 1) * 512],
                    start=(pr == 0), stop=(pr == PAIRS - 1),
                )
            osb = osbpool.tile([P, 512], bf16, tag="osb", name="osb")
            nc.vector.tensor_copy(osb[:], o[:])
            nc.sync.dma_start(
                out_d[(fb * 4 + ft) * P:(fb * 4 + ft + 1) * P,
                      hb * 512:(hb + 1) * 512],
                osb[:],
            )

        for ft in range(4):
            ops.append(lambda ft=ft: transp(ft))
        for ft in range(4):
            for hb in range(H_ // 512):
                ops.append(lambda ft=ft, hb=hb: oproj(ft, hb))
        return ops

    # ---- streaming phase: projections + fb0 live + fb1 exp-ahead ----
    ctx0 = ps_ctx.tile([P, 4, HPG, D + 1], f32, tag="ctx", name="ctx")
    nc.vector.memset(ctx0[:], 0.0)
    ring = {fb: [] for fb in range(1, FB)}  # retained (pt, tp, head)

    proj_chunk(qT_v, wq_sb, qblk, 0)
    nc.sync.dma_start(wk_sb[:], wk_v)
    nc.sync.dma_start(wv_sb[:], wv_v)
    for tb in range(TB):
        schunk = proj_chunk(sT_v, wk_sb, kTp, tb)
        vproj_chunk(schunk, tb)
        if tb + 1 < TB:
            proj_chunk(qT_v, wq_sb, qblk, tb + 1)
        if tb == 0:
            nc.sync.dma_start(wo_sb[:], wo_d[:])
            nc.sync.dma_start(id_sb[:], id_d[:])
        for tp in (2 * tb, 2 * tb + 1):
            for head in range(HPG):
                pt = unit_exp(0, tp, head, ptlive)
                unit_pv(ctx0, pt, tp, head)
            for head in range(HPG):
                ring[1].append((unit_exp(1, tp, head, ptring), tp, head))

    pending = tail_ops(0, ctx0)

    # ---- remaining f-blocks: PV from the ring + exp-ahead for fb+1 ----
    for fb in range(1, FB):
        ctx_ps = ps_ctx.tile([P, 4, HPG, D + 1], f32, tag="ctx", name="ctx")
        nc.vector.memset(ctx_ps[:], 0.0)
        for j, (pt, tp, head) in enumerate(ring[fb]):
            if fb + 1 < FB:
                tpn, headn = divmod(j, HPG)
                ring[fb + 1].append(
                    (unit_exp(fb + 1, tpn, headn, ptring), tpn, headn))
            unit_pv(ctx_ps, pt, tp, head)
            if pending and j % 2 == 1:
                pending.pop(0)()
        while pending:
            pending.pop(0)()
        pending = tail_ops(fb, ctx_ps)
    while pending:
        pending.pop(0)()

    if env.get("debug"):
        nc.sync.dma_start(env["dbg_qblk"][:], qblk[:])
        nc.sync.dma_start(env["dbg_kTp"][:], kTp[:])
        nc.sync.dma_start(env["dbg_vplus"][:], vplus[:])
        nc.sync.dma_start(env["dbg_ctxT"][:], ctxT[:])


def _get_nc():
    if "nc" not in _CACHE:
        _CACHE["nc"] = _build_nc()
    return _CACHE["nc"]


def _bf16(x):
    import ml_dtypes
    return np.ascontiguousarray(x.astype(ml_dtypes.bfloat16))


def _make_in_maps(query_input, source_input, wq, wk, wv, wo):
    qT = [np.ascontiguousarray(query_input[b].T) for b in range(B)]
    sT = [np.ascontiguousarray(source_input[b].T) for b in range(B)]
    ident = _bf16(np.eye(P, dtype=np.float32))
    in_maps = []
    for c in range(NCORES):
        b, g = divmod(c, GROUPS)
        h0 = g * HPG
        # pair-packed weight columns: (pair, h2, d)
        wq_p = wq[:, h0:h0 + HPG, :].reshape(H, HPG * D) * (D ** -0.5)
        wk_p = wk[:, h0:h0 + HPG, :].reshape(H, HPG * D)
        wv_p = wv[:, h0:h0 + HPG, :].reshape(H, HPG * D)
        # wo rows (h2, d), pair-major second dim
        wo_p = wo[h0:h0 + HPG].reshape(PAIRS, 2 * D, H).transpose(1, 0, 2)
        in_maps.append({
            "qT": _bf16(qT[b]),
            "sT": _bf16(sT[b]),
            "wq": _bf16(wq_p),
            "wk": _bf16(wk_p),
            "wv": _bf16(wv_p),
            "wo": _bf16(np.ascontiguousarray(wo_p)),
            "ident": ident,
        })
    return in_maps


def _numpy_fallback(query_input, source_input, bias, wq, wk, wv, wo):
    q = np.einsum("bfd,dnh->bfnh", query_input, wq) * (D ** -0.5)
    k = np.einsum("btd,dnh->btnh", source_input, wk)
    v = np.einsum("btd,dnh->btnh", source_input, wv)
    logits = np.einsum("btnh,bfnh->bnft", k, q) + bias
    logits -= logits.max(axis=-1, keepdims=True)
    w = np.exp(logits)
    w /= w.sum(axis=-1, keepdims=True)
    ctx = np.einsum("bnft,btnh->bfnh", w, v)
    return np.einsum("bfnh,nhd->bfd", ctx, wo).astype(np.float32)


def kernel(query_input, source_input, bias, wq, wk, wv, wo):
    query_input = np.asarray(query_input, np.float32)
    source_input = np.asarray(source_input, np.float32)
    bias = np.asarray(bias, np.float32)
    wq = np.asarray(wq, np.float32)
    wk = np.asarray(wk, np.float32)
    wv = np.asarray(wv, np.float32)
    wo = np.asarray(wo, np.float32)

    if bias.any():
        return _numpy_fallback(query_input, source_input, bias, wq, wk, wv, wo)

    from concourse.bass_utils import run_bass_kernel_spmd

    nc = _get_nc()
    in_maps = _make_in_maps(query_input, source_input, wq, wk, wv, wo)
    last_err = None
    for _attempt in range(3):  # axon tunnel/device hiccups are transient
        try:
            res = run_bass_kernel_spmd(nc, in_maps, core_ids=list(range(NCORES)))
            break
        except Exception as e:  # noqa: BLE001
            last_err = e
            import time as _time
            _time.sleep(5)
    else:
        raise last_err
    parts = [np.asarray(res.results[c]["out"], np.float32)
             for c in range(NCORES)]
    out = np.stack(
        [np.sum(parts[b * GROUPS:(b + 1) * GROUPS], axis=0) for b in range(B)]
    ).astype(np.float32)
    return out


# revision 61
# speedup vs baseline: 1.0570x; 1.0570x over previous
"""Multi-head cross-attention kernel for 8 Trainium2 NeuronCores.

Problem (nn_Attention): B=2, F=T=2048, H=1024, N=16 heads, D=64.
    q = query @ wq;  k = source @ wk;  v = source @ wv     ([B,L,N,D])
    logits = (q * D^-0.5) . k  (+ bias);  w = softmax(logits, T)
    out = (w . v) @ wo                                      ([B,F,H])

Sharding: 8 cores = 2 (batch) x 4 (head groups of 4 heads). Each core
computes its batch's partial output over its 4 heads; the host sums the
4 per-group partials per batch (output projection is linear in heads).

v2 design notes (vs the fp32r baseline):
  - All matmul operands are bf16 (cost model: 1.0 cycles/output-row for
    any N, vs fp32r which needs N>=256).  Accuracy measured ~0.5% absmax
    end-to-end, well under the 2e-2 gate.
  - Logits matmuls run with K=64 directly (cost is independent of K), so
    no zero-padded wq and Q projection is pair-packed: 32768 cycles
    instead of 65536.
  - PV runs in [f-partition, (head,d)-free] orientation: per (head,
    t-tile, f-tile) the matmul is lhsT=pt-slice [128t x 128f], rhs =
    v|1 [128t x 65] -> ctx[128f, 65] in PSUM.  66.5K cycles vs 131K for
    the [d, f] orientation (output rows are full 128).
  - ctx is normalized (per-partition scalar = 1/den from the ones
    column), transposed back to [(h2,d), f] via PE transposes, and the
    output projection accumulates ctxT^T @ wo pairs into [128f, 512h]
    PSUM tiles DMA'd straight to DRAM.
  - exp runs on ScalarE from PSUM [128, 2, 512] tiles (~1.07us each,
    128 per core).  To keep ScalarE busy from ~8us on, logits+exp for
    the NEXT f-block run ahead (pt tiles retained in a 34-deep SBUF
    ring) while PV accumulation stays f-block-major (PSUM ctx capacity
    allows only one f-block's accumulators).

bias is all-zero for this problem (spec fill=zeros); a nonzero bias falls
back to a numpy reference implementation for correctness.
"""

import numpy as np

B, F, T, H, NH, D = 2, 2048, 2048, 1024, 16, 64
NCORES = 8
GROUPS = 4           # head groups (one per core within a batch)
HPG = NH // GROUPS   # 4 heads per core
PAIRS = HPG // 2     # head pairs per core
P = 128

_CACHE = {}


def _build_nc(F_=F, T_=T, H_=H, loop=1, sim_trace=False, skip_compile=False,
              debug=False, bodies=1):
    """Build the per-core Bass program. All 8 cores run this same program
    on different input data. loop>1 repeats the whole body inside the NEFF
    (benchmarking aid: isolates HW time from dispatch overhead)."""
    import concourse.bass as bass  # noqa: F401  (registers engine types)
    import concourse.mybir as mybir
    from concourse import bacc
    from concourse.tile import TileContext

    f32 = mybir.dt.float32
    bf16 = mybir.dt.bfloat16

    nc = bacc.Bacc("TRN2", target_bir_lowering=False, debug=False,
                   num_devices=NCORES)

    qT_d = nc.dram_tensor("qT", [H_, F_], bf16, kind="ExternalInput")
    sT_d = nc.dram_tensor("sT", [H_, T_], bf16, kind="ExternalInput")
    wq_d = nc.dram_tensor("wq", [H_, HPG * D], bf16, kind="ExternalInput")
    wk_d = nc.dram_tensor("wk", [H_, HPG * D], bf16, kind="ExternalInput")
    wv_d = nc.dram_tensor("wv", [H_, HPG * D], bf16, kind="ExternalInput")
    wo_d = nc.dram_tensor("wo", [P, PAIRS, H_], bf16, kind="ExternalInput")
    id_d = nc.dram_tensor("ident", [P, P], bf16, kind="ExternalInput")
    out_d = nc.dram_tensor("out", [F_, H_], bf16, kind="ExternalOutput")

    env = dict(H_=H_, F_=F_, T_=T_, qT_d=qT_d, sT_d=sT_d, wq_d=wq_d,
               wk_d=wk_d, wv_d=wv_d, wo_d=wo_d, id_d=id_d, out_d=out_d,
               debug=debug)
    if debug:
        bf16 = mybir.dt.bfloat16
        env["dbg_qblk"] = nc.dram_tensor("dbg_qblk", [P, PAIRS, F_], bf16,
                                         kind="ExternalOutput")
        env["dbg_kTp"] = nc.dram_tensor("dbg_kTp", [P, PAIRS, T_], bf16,
                                        kind="ExternalOutput")
        env["dbg_vplus"] = nc.dram_tensor("dbg_vplus",
                                          [P, T_ // P, HPG, D + 1], bf16,
                                          kind="ExternalOutput")
        env["dbg_ctxT"] = nc.dram_tensor("dbg_ctxT", [P, PAIRS, F_], bf16,
                                         kind="ExternalOutput")
        env["dbg_ctxsb"] = nc.dram_tensor("dbg_ctxsb",
                                          [F_ // 512, P, 4, HPG, D], bf16,
                                          kind="ExternalOutput")

    with TileContext(nc, trace_sim=sim_trace) as tc:
        with (
            tc.tile_pool(name="weights", bufs=1) as wpool,
            tc.tile_pool(name="persist", bufs=1) as perspool,
            tc.tile_pool(name="stream", bufs=3) as streampool,
            tc.tile_pool(name="ptlive", bufs=8) as ptlive,
            tc.tile_pool(name="ptring", bufs=55) as ptring,
            tc.tile_pool(name="nrm", bufs=2) as nrmpool,
            tc.tile_pool(name="osb", bufs=2) as osbpool,
            tc.tile_pool(name="small", bufs=2) as smallpool,
            tc.tile_pool(name="ps_s", bufs=2, space="PSUM") as ps_s,
            tc.tile_pool(name="ps_ctx", bufs=1, space="PSUM") as ps_ctx,
            tc.tile_pool(name="ps_tail", bufs=1, space="PSUM") as ps_tail,
        ):
            env.update(wpool=wpool, perspool=perspool, streampool=streampool,
                       ptlive=ptlive, ptring=ptring, nrmpool=nrmpool,
                       osbpool=osbpool, smallpool=smallpool, ps_s=ps_s,
                       ps_ctx=ps_ctx, ps_tail=ps_tail)
            import contextlib
            loop_ctx = tc.For_i(0, loop, 1) if loop > 1 else contextlib.nullcontext()
            with loop_ctx:
                for _ in range(bodies):
                    _emit_body(nc, tc, env)

    if not skip_compile:
        nc.compile()
    return nc


def _emit_body(nc, tc, env):
    import concourse.mybir as mybir
    f32 = mybir.dt.float32
    bf16 = mybir.dt.bfloat16
    AF = mybir.ActivationFunctionType
    (H_, F_, T_) = (env[k] for k in ("H_", "F_", "T_"))
    (qT_d, sT_d, wq_d, wk_d, wv_d, wo_d, id_d, out_d) = (env[k] for k in
        ("qT_d", "sT_d", "wq_d", "wk_d", "wv_d", "wo_d", "id_d", "out_d"))
    (wpool, perspool, streampool, ptlive, ptring, nrmpool, osbpool,
     smallpool, ps_s, ps_ctx, ps_tail) = (env[k] for k in
        ("wpool", "perspool", "streampool", "ptlive", "ptring", "nrmpool",
         "osbpool", "smallpool", "ps_s", "ps_ctx", "ps_tail"))

    HT = H_ // P          # contraction tiles for projections (8)
    FB = F_ // 512        # f-blocks (4)
    TB = T_ // 512        # source chunks (4)
    TT = T_ // P          # t-tiles (16)

    qT_v = qT_d[:].rearrange("(o p) f -> p o f", p=P)   # [128, HT, F]
    sT_v = sT_d[:].rearrange("(o p) f -> p o f", p=P)
    wq_v = wq_d[:].rearrange("(o p) c -> p o c", p=P)   # [128, HT, 256]
    wk_v = wk_d[:].rearrange("(o p) c -> p o c", p=P)
    wv_v = wv_d[:].rearrange("(o p) c -> p o c", p=P)

    # ---- persistent SBUF tensors ----
    wq_sb = wpool.tile([P, HT, HPG * D], bf16)   # cols = (pair, h2, d)
    wk_sb = wpool.tile([P, HT, HPG * D], bf16)
    wv_sb = wpool.tile([P, HT, HPG * D], bf16)   # cols = (head, d)
    wo_sb = wpool.tile([P, PAIRS, H_], bf16)     # rows = (h2, d)
    id_sb = wpool.tile([P, P], bf16)

    qblk = perspool.tile([P, PAIRS, F_], bf16)   # pair-packed q^T
    kTp = perspool.tile([P, PAIRS, T_], bf16)    # pair-packed k^T
    vplus = perspool.tile([P, TT, HPG, D + 1], bf16)  # [t%128, tt, h, d|1]
    ctxT = perspool.tile([P, PAIRS, F_], bf16)   # normalized ctx^T

    nc.vector.memset(vplus[:, :, :, D:D + 1], 1.0)

    nc.sync.dma_start(wq_sb[:], wq_v)

    # PE p-state warm-up: the tensor engine needs ~3us of continuous work
    # to reach 2.4GHz (0.65/1.2GHz below that).  Dummy matmuls on a zeroed
    # scratch keep it busy while the first input DMAs land, so the real
    # projections start at full clock.
    warm_sb = wpool.tile([P, P], bf16, name="warm_sb")
    nc.vector.memset(warm_sb[:], 0.0)
    warm_ps = ps_tail.tile([P, 512], f32, tag="tail", name="warm_ps")
    for _ in range(14):
        nc.tensor.matmul(warm_ps[:, 0:P], warm_sb[:, 0:P], warm_sb[:],
                         start=True, stop=True)

    def proj_chunk(src_v, w_sb, dst, c, col0=0, width=512):
        """Project one chunk of qT/sT through wq/wk (pair-packed M=128
        output tiles) into dst[:, :, c*512+col0 : +width]."""
        lo = c * 512 + col0
        chunk = streampool.tile([P, HT, width], bf16, tag="chunk",
                                name="chunk", padded_shape=(P, HT, 512))
        nc.sync.dma_start(chunk[:], src_v[:, :, lo:lo + width])
        ps = ps_s.tile([P, 2, 512], f32, tag="s", name="ps_proj")
        for pair in range(PAIRS):
            for ht in range(HT):
                nc.tensor.matmul(
                    ps[:, pair, 0:width],
                    w_sb[:, ht, pair * P:(pair + 1) * P],
                    chunk[:, ht, :],
                    start=(ht == 0), stop=(ht == HT - 1),
                )
        nc.vector.tensor_copy(dst[:, :, lo:lo + width], ps[:, :, 0:width])
        return chunk

    def vproj_chunk(schunk, tb):
        """sourceT chunk -> v tiles [t, (h,d)] written into vplus.
        schunk may be a (halfA, halfB) pair of 256-wide tiles."""
        ps = ps_s.tile([P, 2, 512], f32, tag="s", name="ps_v")
        for tc4 in range(4):
            if isinstance(schunk, tuple):
                src, tcol = schunk[tc4 // 2], (tc4 % 2) * P
            else:
                src, tcol = schunk, tc4 * P
            for ht in range(HT):
                nc.tensor.matmul(
                    ps[:, tc4 // 2, (tc4 % 2) * 256:(tc4 % 2 + 1) * 256],
                    src[:, ht, tcol:tcol + P],
                    wv_sb[:, ht, :],
                    start=(ht == 0), stop=(ht == HT - 1),
                )
        nc.vector.tensor_copy(
            vplus[:, tb * 4:(tb + 1) * 4, :, 0:D],
            ps[:].rearrange("p a (b h d) -> p (a b) h d", b=2, h=HPG),
        )

    def unit_exp(fb, tp, head, pool):
        """Logits (2 t-tiles) + exp for one (f-block, t-pair, head).
        Returns the pt tile holding exp(S^T) [128t, 2, 512f]."""
        pair, h2 = divmod(head, 2)
        lo = 64 * h2
        s = ps_s.tile([P, 2, 512], f32, tag="s", name="s")
        for i in range(2):
            tt = 2 * tp + i
            nc.tensor.matmul(
                s[:, i, :],
                kTp[lo:lo + 64, pair, tt * P:(tt + 1) * P],
                qblk[lo:lo + 64, pair, fb * 512:(fb + 1) * 512],
                start=True, stop=True,
            )
        pt = pool.tile([P, 2, 512], bf16, tag="pt", name="pt")
        nc.scalar.activation(pt[:], s[:], AF.Exp)
        return pt

    def make_ctx():
        """ctx accumulator [128f, ft, h, d|den] in the dedicated PSUM
        bank group, as a single part."""
        t = ps_ctx.tile([P, 4, HPG, D + 1], f32, tag="ctx", name="ctx")
        nc.vector.memset(t[:], 0.0)
        return [(t, 0, 4)]

    def make_ctx_s():
        """ctx accumulator for the last f-block carved out of two s-pool
        tiles (free of logits by then), so its PV can overlap the
        previous f-block's."""
        parts = []
        for i in range(2):
            t = ps_s.tile([P, 2, 512], f32, tag="s", name="ctx3")
            v = t.rearrange("p a b -> p (a b)")[:, 0:2 * HPG * (D + 1)]
            v = v.rearrange("p (f h e) -> p f h e", f=2, h=HPG)
            nc.vector.memset(v[:], 0.0)
            parts.append((v, 2 * i, 2))
        return parts

    def ctx_at(parts, ft, head):
        for ap, ft0, nft in parts:
            if ft0 <= ft < ft0 + nft:
                return ap[:, ft - ft0, head, :]
        raise AssertionError(ft)

    def unit_pv(parts, pt, tp, head):
        """Accumulate pt into an f-block's ctx PSUM for one unit.

        start is never set: 16 accumulation groups (ft x head) share the
        ctx PSUM banks, and a start_tensor_calc marks its whole 2KB
        ZERO_REGION pending-zero, wiping other groups' partials.  The
        tile is DVE-memset to zero once per f-block instead."""
        for i in range(2):
            tt = 2 * tp + i
            for ft in range(4):
                nc.tensor.matmul(
                    ctx_at(parts, ft, head),
                    pt[:, i, ft * P:(ft + 1) * P],
                    vplus[:, tt, head, :],
                    start=False, stop=(tt == TT - 1),
                    skip_group_check=True,
                )

    def tail_ops(fb, parts):
        """Normalize + transpose + output projection for a finished
        f-block. DVE normalize is emitted inline; returns a list of
        closures (PE transposes / oproj / copies) to interleave into the
        next phase so single-buffer PSUM reuse stalls overlap real work.
        The last f-blocks' tails borrow the (by then idle) s-pool PSUM
        banks for double-buffering instead of the single tail bank."""
        from concourse.bass import broadcast_tensor_aps
        recip = smallpool.tile([P, 4, HPG, 1], f32, tag="recip", name="recip")
        for ap, ft0, nft in parts:
            nc.vector.reciprocal(recip[:, ft0:ft0 + nft],
                                 ap[:, :, :, D:D + 1])
        ctx_sb = nrmpool.tile([P, 4, HPG, D], bf16, tag="nrm", name="ctx_sb")
        for ap, ft0, nft in parts:
            for f in range(nft):
                in0, in1 = broadcast_tensor_aps(
                    ap[:, f, :, 0:D], recip[:, ft0 + f, :, :])
                nc.vector.tensor_mul(ctx_sb[:, ft0 + f, :, :], in0, in1)
        if env.get("debug"):
            nc.sync.dma_start(env["dbg_ctxsb"][fb], ctx_sb[:])
        last = (fb == FB - 1)

        def tail_tile():
            if last:
                t = ps_s.tile([P, 2, 512], f32, tag="s", name="tail_s")
                return t[:, 0, :], t.bitcast(bf16)
            t = ps_tail.tile([P, 512], f32, tag="tail", name="tail")
            return t[:], t.bitcast(bf16).rearrange("p (a b) -> p a b", a=2)

        ops = []

        def transp(ft):
            _, tlb = tail_tile()
            for h in range(HPG):
                pr, h2 = divmod(h, 2)
                nc.tensor.transpose(
                    tlb[64 * h2:64 * (h2 + 1), pr, 0:P],
                    ctx_sb[:, ft, h, :],
                    id_sb[:],
                    tile_position=(0, 64 * h2),
                )
            nc.vector.tensor_copy(
                ctxT[:, :, (fb * 4 + ft) * P:(fb * 4 + ft + 1) * P],
                tlb[:, :, 0:P],
            )

        def oproj(ft, hb):
            o, _ = tail_tile()
            for pr in range(PAIRS):
                nc.tensor.matmul(
                    o,
                    ctxT[:, pr, (fb * 4 + ft) * P:(fb * 4 + ft + 1) * P],
                    wo_sb[:, pr, hb * 512:(hb + 1) * 512],
                    start=(pr == 0), stop=(pr == PAIRS - 1),
                )
            osb = osbpool.tile([P, H_], bf16, tag="osb", name="osb")[:, 0:512]
            if fb >= FB - 2 and (ft + hb) % 2 == 0:
                # ScalarE is idle by then; split copies across engines
                nc.scalar.copy(osb[:], o)
            else:
                nc.vector.tensor_copy(osb[:], o)
            nc.sync.dma_start(
                out_d[(fb * 4 + ft) * P:(fb * 4 + ft + 1) * P,
                      hb * 512:(hb + 1) * 512],
                osb[:],
            )

        def oproj_wide(ft):
            # both 512-wide halves into one 2-bank s-pool tile: one copy
            # and one DMA per f-tile (fewer serial hops at the kernel end)
            t = ps_s.tile([P, 2, 512], f32, tag="s", name="tail_s")
            for hb in range(H_ // 512):
                for pr in range(PAIRS):
                    nc.tensor.matmul(
                        t[:, hb, :],
                        ctxT[:, pr, (fb * 4 + ft) * P:(fb * 4 + ft + 1) * P],
                        wo_sb[:, pr, hb * 512:(hb + 1) * 512],
                        start=(pr == 0), stop=(pr == PAIRS - 1),
                    )
            osb = osbpool.tile([P, H_], bf16, tag="osb", name="osbw")
            if ft % 2 == 0:
                nc.scalar.copy(osb[:], t[:])
            else:
                nc.vector.tensor_copy(osb[:], t[:])
            nc.sync.dma_start(
                out_d[(fb * 4 + ft) * P:(fb * 4 + ft + 1) * P, :], osb[:])

        for ft in range(4):
            ops.append(lambda ft=ft: transp(ft))
        if last:
            for ft in range(4):
                ops.append(lambda ft=ft: oproj_wide(ft))
        else:
            for ft in range(4):
                for hb in range(H_ // 512):
                    ops.append(lambda ft=ft, hb=hb: oproj(ft, hb))
        return ops

    # ---- streaming phase: projections + fb0 live, exp-ahead for fb1
    # (all 4 chunks) and fb2 (chunks 1-3, one chunk behind) so ScalarE
    # stays fed while the PE streams K/V/Q projections. ----
    ctx0 = make_ctx()
    ring = {fb: [] for fb in range(1, FB)}  # retained (pt, tp, head)

    # Prelude: Q/K projections for chunk 0 (DMA queue: wq, qc0, wk,
    # sc0a, sc0b).  The K side streams in two halves so the first
    # logits+exp fire ~4us earlier.
    proj_chunk(qT_v, wq_sb, qblk, 0)
    nc.sync.dma_start(wk_sb[:], wk_v)
    kchunks = {0: (proj_chunk(sT_v, wk_sb, kTp, 0, 0, 256),
                   proj_chunk(sT_v, wk_sb, kTp, 0, 256, 256))}

    def filler_q(c):
        return lambda: proj_chunk(qT_v, wq_sb, qblk, c)

    def filler_k(c):
        return lambda: kchunks.__setitem__(
            c, proj_chunk(sT_v, wk_sb, kTp, c))

    def filler_v(tb):
        def f():
            if tb == 0:
                nc.sync.dma_start(wv_sb[:], wv_v)
            vproj_chunk(kchunks[tb], tb)
        return f

    def filler_w():
        def f():
            nc.sync.dma_start(wo_sb[:], wo_d[:])
            nc.sync.dma_start(id_sb[:], id_d[:])
        return f

    # Per chunk tb: 16 fresh exps (fb0+fb1, both t-pairs) + 8 lagged fb2
    # exps feed ScalarE; the next chunk's projections slot between the
    # 4-unit groups (after the q-chunk each group needs is projected).
    fillers = {
        0: [filler_q(1), filler_v(0), filler_k(1)],
        1: [filler_q(2), filler_v(1), filler_k(2), filler_w()],
        2: [filler_q(3), filler_v(2), filler_k(3)],
        3: [filler_v(3)],
    }
    for tb in range(TB):
        fl = fillers[tb]
        pts0 = []
        for gi, (fb, tp) in enumerate(
                [(0, 2 * tb), (0, 2 * tb + 1), (1, 2 * tb), (1, 2 * tb + 1)]):
            for head in range(HPG):
                if fb == 0:
                    pts0.append((tp, head, unit_exp(0, tp, head, ptlive)))
                else:
                    ring[1].append((unit_exp(1, tp, head, ptring), tp, head))
            if gi < len(fl):
                fl[gi]()
        for tp, h, pt in pts0:
            unit_pv(ctx0, pt, tp, h)
        # lagged fb2 units drain into the next chunk's K-proj window
        if tb >= 1:
            for tp in (2 * tb - 2, 2 * tb - 1):
                for head in range(HPG):
                    ring[2].append((unit_exp(2, tp, head, ptring), tp, head))

    pending = tail_ops(0, ctx0)

    # ---- fb1 phase: PV from the ring, exp-ahead for fb2's last chunk
    # and fb3's chunks 0-5 (one ring slot frees per consumed unit). ----
    plan1 = ([(2, tp, h) for tp in (6, 7) for h in range(HPG)]
             + [(3, tp, h) for tp in range(6) for h in range(HPG)])
    ctx1 = make_ctx()
    for j, (pt, tp, head) in enumerate(ring[1]):
        fbn, tpn, hn = plan1[j]
        ring[fbn].append((unit_exp(fbn, tpn, hn, ptring), tpn, hn))
        unit_pv(ctx1, pt, tp, head)
        if pending and j % 2 == 1:
            pending.pop(0)()
    while pending:
        pending.pop(0)()
    pending = tail_ops(1, ctx1)

    # ---- fused fb2+fb3 phase: fb2 accumulates in the ctx banks while
    # fb3 accumulates into two repurposed s-pool tiles, so the final
    # f-block's PV overlaps fb2's instead of serializing after it. ----
    ctx2 = make_ctx()
    for j in range(8):
        tpn, hn = divmod(j, HPG)
        tpn += 6
        ring[3].append((unit_exp(3, tpn, hn, ptring), tpn, hn))
        pt, tp, head = ring[2][j]
        unit_pv(ctx2, pt, tp, head)
        if pending:
            pending.pop(0)()
    ctx3 = make_ctx_s()
    for j in range(8, 32):
        pt, tp, head = ring[2][j]
        unit_pv(ctx2, pt, tp, head)
        pt, tp, head = ring[3][j - 8]
        unit_pv(ctx3, pt, tp, head)
        if pending:
            pending.pop(0)()
    while pending:
        pending.pop(0)()
    pend2 = tail_ops(2, ctx2)
    for j in range(24, 32):
        pt, tp, head = ring[3][j]
        unit_pv(ctx3, pt, tp, head)
        if pend2 and j % 2 == 1:
            pend2.pop(0)()
    pend3 = tail_ops(3, ctx3)
    while pend2 or pend3:
        if pend2:
            pend2.pop(0)()
        if pend3:
            pend3.pop(0)()

    if env.get("debug"):
        nc.sync.dma_start(env["dbg_qblk"][:], qblk[:])
        nc.sync.dma_start(env["dbg_kTp"][:], kTp[:])
        nc.sync.dma_start(env["dbg_vplus"][:], vplus[:])
        nc.sync.dma_start(env["dbg_ctxT"][:], ctxT[:])


def _get_nc():
    if "nc" not in _CACHE:
        _CACHE["nc"] = _build_nc()
    return _CACHE["nc"]


def _bf16(x):
    import ml_dtypes
    return np.ascontiguousarray(x.astype(ml_dtypes.bfloat16))


def _make_in_maps(query_input, source_input, wq, wk, wv, wo):
    qT = [np.ascontiguousarray(query_input[b].T) for b in range(B)]
    sT = [np.ascontiguousarray(source_input[b].T) for b in range(B)]
    ident = _bf16(np.eye(P, dtype=np.float32))
    in_maps = []
    for c in range(NCORES):
        b, g = divmod(c, GROUPS)
        h0 = g * HPG
        # pair-packed weight columns: (pair, h2, d)
        wq_p = wq[:, h0:h0 + HPG, :].reshape(H, HPG * D) * (D ** -0.5)
        wk_p = wk[:, h0:h0 + HPG, :].reshape(H, HPG * D)
        wv_p = wv[:, h0:h0 + HPG, :].reshape(H, HPG * D)
        # wo rows (h2, d), pair-major second dim
        wo_p = wo[h0:h0 + HPG].reshape(PAIRS, 2 * D, H).transpose(1, 0, 2)
        in_maps.append({
            "qT": _bf16(qT[b]),
            "sT": _bf16(sT[b]),
            "wq": _bf16(wq_p),
            "wk": _bf16(wk_p),
            "wv": _bf16(wv_p),
            "wo": _bf16(np.ascontiguousarray(wo_p)),
            "ident": ident,
        })
    return in_maps


def _numpy_fallback(query_input, source_input, bias, wq, wk, wv, wo):
    q = np.einsum("bfd,dnh->bfnh", query_input, wq) * (D ** -0.5)
    k = np.einsum("btd,dnh->btnh", source_input, wk)
    v = np.einsum("btd,dnh->btnh", source_input, wv)
    logits = np.einsum("btnh,bfnh->bnft", k, q) + bias
    logits -= logits.max(axis=-1, keepdims=True)
    w = np.exp(logits)
    w /= w.sum(axis=-1, keepdims=True)
    ctx = np.einsum("bnft,btnh->bfnh", w, v)
    return np.einsum("bfnh,nhd->bfd", ctx, wo).astype(np.float32)


def kernel(query_input, source_input, bias, wq, wk, wv, wo):
    query_input = np.asarray(query_input, np.float32)
    source_input = np.asarray(source_input, np.float32)
    bias = np.asarray(bias, np.float32)
    wq = np.asarray(wq, np.float32)
    wk = np.asarray(wk, np.float32)
    wv = np.asarray(wv, np.float32)
    wo = np.asarray(wo, np.float32)

    if bias.any():
        return _numpy_fallback(query_input, source_input, bias, wq, wk, wv, wo)

    from concourse.bass_utils import run_bass_kernel_spmd

    nc = _get_nc()
    in_maps = _make_in_maps(query_input, source_input, wq, wk, wv, wo)
    last_err = None
    for _attempt in range(3):  # axon tunnel/device hiccups are transient
        try:
            res = run_bass_kernel_spmd(nc, in_maps, core_ids=list(range(NCORES)))
            break
        except Exception as e:  # noqa: BLE001
            last_err = e
            import time as _time
            _time.sleep(5)
    else:
        raise last_err
    parts = [np.asarray(res.results[c]["out"], np.float32)
             for c in range(NCORES)]
    out = np.stack(
        [np.sum(parts[b * GROUPS:(b + 1) * GROUPS], axis=0) for b in range(B)]
    ).astype(np.float32)
    return out
